# revision 60
# baseline (speedup 1.0000x reference)
"""CodaPrompt kernel for Trainium2 (Bass/Tile) on 8 NeuronCores.

Math (reference):
    a[e,b,k,:] = x[b,:] * As[e,k,:]
    q = a / max(||a||_2, eps)        (normalize over d)
    nK = Ks / max(||Ks||_2, eps)
    aq[e,b,k] = <q[e,b,k,:], nK[e,k,:]>
    P_[e,b,l,:] = sum_k aq[e,b,k] * Ps[e,k,l,:]
    out = stack([P_[:,:, :L/2], P_[:,:, L/2:]])   # [2, E, B, L/2, D]

Sharding: SSPLIT L-slices x (8/SSPLIT) batch-slices (default 2x4). The cost
model serializes ALL DMA on one 360GB/s device, so total bytes/core is the
roofline; the output store dominates (15.7MB in bf16 = 42.7us of the ~58us
total). SSPLIT=2 balances the Ps input duplication (6.14MB/SSPLIT) against
num/den compute duplication (scales with B/QSPLIT): PE work (38us) stays
safely under the DMA floor so the store stream is never PE-paced.

Timeline shape: ~2us DMA pipeline latency, ~14us input loads (during which
PE warms its p-state on dummy matmuls and runs num/den for e=0; x^2 splits
DVE/ACT so neither serial chain gates den), then 60 back-to-back 728ns
output stores (2 psum chunks each — small enough to issue right behind
their own copies, large enough to stay above the ~650ns HWDGE descriptor-
gen floor) with num/den for e+1 interleaved between the P_ store groups of
e. psum->sbuf copies round-robin ACT/DVE (GPSIMD cannot access PSUM).
Measured: 61363ns = 57.6us serial DMA + 2.0us DMA pipeline startup +
1.5us completion-sem tail + ~0.3us slack. Verified HW rel err: 5.7e-3
(gate 2e-2).

Device-side formulation (per core: batch slice of BC rows, one L-slice):
    num[e,k,b] = sum_d (As*nK)[e,k,d] * x[b,d]        -> matmul, contraction over d
    den2[e,k,b] = sum_d (As*As)[e,k,d] * x2[b,d]      -> matmul (x2 on device)
    aq[e,k,b] = num * rsqrt(den2)                      (ACT sqrt + DVE recip + mul)
    P_half[b, (l d)] = aq[e,:,b].T @ Ps[e, :, half]    -> matmul, contraction over k

Host prep is O(E*K*D) pool preprocessing (normalize Ks, fuse/transpose
weights, slice Ps halves) plus the x transpose; all O(B*...) FLOPs on device.
"""

import os
import sys
from contextlib import ExitStack

import numpy as np

if "/opt/trn_rl_repo" not in sys.path:
    sys.path.insert(0, "/opt/trn_rl_repo")

import concourse.mybir as mybir
from concourse import bacc, tile
from concourse.bass_utils import run_bass_kernel_spmd

B, D, E, K, L = 2048, 768, 5, 100, 8
NCORES = 8
SSPLIT = int(os.environ.get("CODA_SSPLIT", "2"))  # L-axis splits (2 or 4)
QSPLIT = NCORES // SSPLIT # batch splits
BC = B // QSPLIT          # batch rows per core
LH = L // SSPLIT          # l entries per core
DC = D // 128             # 6 contraction chunks of 128
NDH = LH * D              # P_ cols per core
NCHUNK = 512              # psum bank width in f32
NJ = NDH // NCHUNK        # n-chunks per core
MC = BC // 128            # output-partition chunks
MSTORE = int(os.environ.get("CODA_MSTORE", "1"))  # m-chunks per output DMA
JSTORE = int(os.environ.get("CODA_JSTORE", "2"))  # j-chunks per store (MSTORE=1)
NWARM = int(os.environ.get("CODA_NWARM", "40"))   # PE clock-ramp dummy matmuls
AQC = int(os.environ.get("CODA_AQC", "512"))      # aq math column-chunk width
AQC0 = int(os.environ.get("CODA_AQC0", "512"))    # aq chunk width for e=0 (head)
AQORD = int(os.environ.get("CODA_AQORD", "0"))    # 0: sqrt->recip; 1: recip->sqrt
AQPRIO = int(os.environ.get("CODA_AQPRIO", "0"))  # scheduler priority hoist for aq ops
# fp8 DoubleRow den matmuls (0.5 cyc/row): compiles with K padded to 128
# (dual-fp8 ldweights ISA wants full/aligned column blocks) but the pad's
# extra DMA bytes outweigh the PE saving at this sharding — PE is not the
# bottleneck. Kept as an option for PE-bound variants.
DROW = int(os.environ.get("CODA_DROW", "0"))
NB = max(1, BC // 512)    # moving-operand chunks for num/den (fp32 N<=512)
EPS = 1e-12

F32 = mybir.dt.float32
# The cost model serializes ALL DMA traffic on one exclusive DMA_ENGINES
# device at 360 GB/s (descriptors/16 * elem_bytes/22.5ns), so total bytes
# moved per core IS the roofline. bf16 end-to-end (inputs, matmuls, output
# store, fp32 upcast on host) halves every byte vs fp32/fp32r at identical
# PE cost (1 cycle/row for both bf16 and fp32r at N>=256); measured
# ~2e-3 scale-relative error vs the 2e-2 gate.
MM_DTYPE = os.environ.get("CODA_MM_DTYPE", "bfloat16")
MM_DT = getattr(mybir.dt, MM_DTYPE)
PS_DTYPE = os.environ.get("CODA_PS_DTYPE", MM_DTYPE)
PS_DT = getattr(mybir.dt, PS_DTYPE)
# Output DRAM tensor dtype: bf16 halves the dominant store stream
# (31.5MB -> 15.7MB per core); host upcasts to fp32 after gather.
OUT_DTYPE = os.environ.get("CODA_OUT_DTYPE", "bfloat16")
OUT_DT = getattr(mybir.dt, OUT_DTYPE)
# den = sum_d (As^2)[d] * (x^2)[d] is an all-positive contraction, so fp8
# quantization error averages down (~0.2% on den, ~0.1% on aq after rsqrt):
# carrying W2=As^2 in fp8e4m3 (x^2 is squared on device into fp8) halves
# that weight stream's DMA for negligible error.
W2_DTYPE = os.environ.get("CODA_W2_DTYPE", "float8e4")
W2_DT = getattr(mybir.dt, W2_DTYPE)


def _build_bass(repeat=1):
    # Bacc (not plain Bass): its finalize() runs move_matmul_waits_to_ldweights
    # + generate_event_semaphores, without which multi-dependency matmuls hit
    # walrus "Too many sync wait commands".
    # `repeat` replicates the whole compute body (timing instrumentation:
    # slope over repeat removes per-launch overhead); results are idempotent.
    nc = bacc.Bacc(None)

    # Matmul operands must be produced as MM_DT end-to-end (walrus verifies
    # fp32r consumers see fp32r producers). float32r is bit-identical to
    # float32 in DRAM, so host arrays stay np.float32 either way.
    xT_d = nc.declare_dram_parameter("xT", [D, BC], MM_DT, isOutput=False)
    # w1/w2 pre-packed on host into SBUF partition-major layout so each
    # loads as ONE full-rate DMA (per-partition runs of 6000B / 3000B).
    w1_d = nc.declare_dram_parameter("w1", [128, DC, E, K], MM_DT, isOutput=False)
    # K padded to 128 when DROW: dual-fp8 ldweights ISA restrictions want
    # full/aligned column blocks (zero columns land in den rows 100..127,
    # which nothing reads).
    KP = 128 if DROW else K
    w2_d = nc.declare_dram_parameter("w2", [128, DC, E, KP], W2_DT, isOutput=False)
    ps_d = nc.declare_dram_parameter("ps", [E, K, NDH], PS_DT, isOutput=False)
    out_d = nc.declare_dram_parameter("out", [E, BC, LH, D], OUT_DT, isOutput=True)

    with ExitStack() as ctx:
        tc = ctx.enter_context(tile.TileContext(nc))
        const = ctx.enter_context(tc.tile_pool(name="const", bufs=1))
        psp = ctx.enter_context(tc.tile_pool(name="psp", bufs=E))
        smallp = ctx.enter_context(tc.tile_pool(name="smallp", bufs=2))
        resp = ctx.enter_context(tc.tile_pool(name="resp", bufs=6))
        # num/den psum tiles span ceil(BC*4B/2KB) banks; keep total <= 8.
        pndp = ctx.enter_context(
            tc.tile_pool(name="pndp", bufs=(2 if BC <= 512 else 1), space="PSUM")
        )
        ppp = ctx.enter_context(tc.tile_pool(name="ppp", bufs=4, space="PSUM"))

        # Resident operands: x slice (transposed) and the fused W1=As*nK /
        # W2=As^2 weight block, chunked to 128 partitions. Per-chunk loads so
        # the first num/den matmuls start as soon as their own d-chunk lands.
        # x^2 is computed on-device (saves its DMA).
        xT_r = xT_d[:].rearrange("(c p) b -> p c b", p=128)
        xs = const.tile([128, DC, BC], MM_DT, name="xs", tag="xs")
        x2s = const.tile([128, DC, BC], W2_DT, name="x2s", tag="x2s")
        ws1 = const.tile([128, DC, E, K], MM_DT, name="ws1", tag="ws1")
        ws2 = const.tile([128, DC, E, KP], W2_DT, name="ws2", tag="ws2")
        # PE p-state warmup: the cost model charges matmuls ~2x cycles until
        # the PE has been continuously busy ~3us. Dummy matmuls on a zeroed
        # scratch tile ramp the clock while the first loads are in flight,
        # so the real num/den matmuls all run at full rate. Results land in
        # rotating psum scratch that is never read.
        if NWARM:
            warm = const.tile([128, 128], MM_DT, name="warm", tag="warm")
            nc.gpsimd.memset(warm[:], 0)
            for _ in range(NWARM):
                wp = ppp.tile([128, NCHUNK], F32, name="pp", tag="pp")
                nc.tensor.matmul(wp[:, :128], warm[:], warm[:], start=True, stop=True)

        # Few, large loads: every transfer stays above the ~650ns HWDGE
        # descriptor-gen serialization (per-chunk ws/xs loads leak ~225ns of
        # dead DMA time each). ws1 in two halves so the first num matmuls
        # start while the second half is in flight.
        nc.sync.dma_start(xs[:], xT_r[:])
        for c0 in range(0, DC, 2):
            nc.sync.dma_start(ws1[:, c0 : c0 + 2], w1_d[:, c0 : c0 + 2])
        nc.sync.dma_start(ws2[:], w2_d[:])
        # x^2 split DVE/ACT: the fp8 output disables DVE's 2x mode (594ns
        # per chunk), and a serial 6-chunk chain on one engine gates den(e0).
        # Both engines are idle during the load phase.
        for c in range(DC):
            if c % 2 == 0:
                nc.vector.tensor_mul(x2s[:, c], xs[:, c], xs[:, c])
            else:
                nc.scalar.square(x2s[:, c], xs[:, c])

        # psum->sbuf copy engines round-robin DVE / ACT. (GPSIMD/Pool cannot
        # access PSUM on TRN2 — the walrus verifier rejects it.) The copies
        # are one full pass over the output; the 2-way split keeps both
        # engines under the serial-DMA floor.
        copy_engines = os.environ.get("CODA_COPY_ENGINES", "sv")
        copy_ctr = [0]

        def emit_copy(dst, src):
            eng = copy_engines[copy_ctr[0] % len(copy_engines)]
            copy_ctr[0] += 1
            if eng == "v":
                nc.vector.tensor_copy(dst, src)
            else:
                nc.scalar.copy(dst, src)

        for _ in range(repeat):
            # All pool loads issue upfront (own slots, bufs=E) so no load
            # ever queues behind output stores in a DMA FIFO. (Staggering the
            # last loads to fill the store-rampup bubble does not work: the
            # scheduler hoists dependency-free DMAs.)
            psts = []
            for e in range(E):
                pst = psp.tile([K, NDH], PS_DT, name="pst", tag="ps")
                nc.sync.dma_start(pst[:], ps_d[e])
                psts.append(pst)

            def make_nd(e, aqc=AQC):
                """aq tile + list of emit-steps (num+den matmuls per d-chunk;
                aq math attached after the last chunk). `aqc` sets the aq
                column-chunk width: fine chunks for e=0 cut the first-store
                latency (P_ m0 only needs aq[:, :128]); coarse elsewhere
                avoids per-op overhead."""
                num = pndp.tile([K, BC], F32, name="num", tag="num")
                den = pndp.tile([KP, BC], F32, name="den", tag="den")
                sden = smallp.tile([K, BC], F32, name="sden", tag="sden")
                rden = smallp.tile([K, BC], F32, name="rden", tag="rden")
                aq = smallp.tile([K, BC], PS_DT, name="aq", tag="aq", bufs=2)
                steps = []
                for nb in range(NB):
                    bsl = slice(nb * 512, min((nb + 1) * 512, BC))

                    def mk(c, bsl=bsl, last=False):
                        # num+den for one d-chunk per step: den finishes right
                        # behind the last weight-chunk load instead of a full
                        # num pass later (shorter first-aq critical path).
                        def emit():
                            nc.tensor.matmul(
                                num[:, bsl],
                                ws1[:, c, e, :],
                                xs[:, c, bsl],
                                start=(c == 0),
                                stop=(c == DC - 1),
                            )
                            if DROW:
                                # fp8 DoubleRow: 2 d-chunks per pass at 0.5
                                # cycles/row — the [128, DC, ...] tiles put
                                # the k-tile pair exactly at AP dim 1.
                                if c % 2 == 1:
                                    nc.tensor.matmul(
                                        den[:, bsl],
                                        ws2[:, c - 1 : c + 1, e, :],
                                        x2s[:, c - 1 : c + 1, bsl],
                                        start=(c == 1),
                                        stop=(c == DC - 1),
                                        perf_mode=mybir.MatmulPerfMode.DoubleRow,
                                    )
                            else:
                                nc.tensor.matmul(
                                    den[:, bsl],
                                    ws2[:, c, e, :],
                                    x2s[:, c, bsl],
                                    start=(c == 0),
                                    stop=(c == DC - 1),
                                )
                            if last:
                                # aq = num / sqrt(den2) (den2 >> eps^2 here),
                                # in column chunks: P_ for m-chunk q only
                                # needs aq[:, q*128:...], so finer chunks cut
                                # the first-store latency and the e-boundary
                                # stall where P_(e+1) waits on the aq chain.
                                b0, b1 = bsl.start, bsl.stop
                                ctx2 = (
                                    tc.high_priority(offset=AQPRIO)
                                    if AQPRIO
                                    else None
                                )
                                if ctx2:
                                    ctx2.__enter__()
                                for q0 in range(b0, b1, aqc):
                                    qsl = slice(q0, min(q0 + aqc, b1))
                                    if AQORD == 0:
                                        nc.scalar.sqrt(sden[:, qsl], den[:K, qsl])
                                        nc.vector.reciprocal(rden[:, qsl], sden[:, qsl])
                                    else:
                                        # same math, DVE->ACT->DVE shape:
                                        # rsqrt(den) = sqrt(1/den)
                                        nc.vector.reciprocal(sden[:, qsl], den[:K, qsl])
                                        nc.scalar.sqrt(rden[:, qsl], sden[:, qsl])
                                    nc.vector.tensor_mul(
                                        aq[:, qsl], num[:, qsl], rden[:, qsl]
                                    )
                                if ctx2:
                                    ctx2.__exit__(None, None, None)

                        return emit

                    for c in range(DC):
                        steps.append(mk(c, last=(c == DC - 1)))
                return aq, steps

            def make_pgroups(e, aq):
                """MC emit-steps: NJ P_ matmuls + copies per m-chunk; one DMA
                per MSTORE m-chunks (fewer, larger stores -> fewer HWDGE gens
                and completion-sem boundaries on the store stream)."""
                pst = psts[e]
                out_r = out_d[e].rearrange("b l d -> b (l d)")
                pgs = []
                res_holder = [None]
                for m in range(MC):
                    def emit(m=m):
                        g = m % MSTORE
                        if g == 0:
                            res_holder[0] = resp.tile(
                                [128, MSTORE * NDH], OUT_DT, name="res", tag="res"
                            )
                        res = res_holder[0]
                        # JSTORE j-chunks per store DMA: finer stores issue
                        # as soon as their own copies land (shorter first-
                        # store latency, smoother stream); transfers must
                        # stay >= the ~650ns HWDGE gen time.
                        for j in range(NJ):
                            pp = ppp.tile([128, NCHUNK], F32, name="pp", tag="pp")
                            nc.tensor.matmul(
                                pp[:],
                                aq[:, m * 128 : (m + 1) * 128],
                                pst[:, j * NCHUNK : (j + 1) * NCHUNK],
                                start=True,
                                stop=True,
                            )
                            emit_copy(
                                res[:, g * NDH + j * NCHUNK : g * NDH + (j + 1) * NCHUNK],
                                pp[:],
                            )
                            if MSTORE == 1 and (j + 1) % JSTORE == 0:
                                j0 = j + 1 - JSTORE
                                nc.sync.dma_start(
                                    out_r[m * 128 : (m + 1) * 128][
                                        :, j0 * NCHUNK : (j + 1) * NCHUNK
                                    ],
                                    res[:, j0 * NCHUNK : (j + 1) * NCHUNK],
                                )
                        if MSTORE > 1 and g == MSTORE - 1:
                            m0 = m - g
                            dst = out_r[m0 * 128 : (m0 + MSTORE) * 128].rearrange(
                                "(g p) n -> p g n", p=128
                            )
                            nc.sync.dma_start(dst, res[:].rearrange(
                                "p (g n) -> p g n", g=MSTORE
                            ))

                    pgs.append(emit)
                return pgs

            # Software pipeline: num/den for e+1 interleaves with the P_
            # store-groups of e, so PE keeps feeding the store stream instead
            # of stalling DMA for ~5us per layer during the nd phase. The
            # interleave is front-biased (factor 2): nd work lands while the
            # store buffer is still full, leaving a pure-P_ tail that streams
            # stores at full DMA rate.
            aq_cur, nd_steps = make_nd(0, aqc=AQC0)
            for s in nd_steps:
                s()
            for e in range(E):
                pgs = make_pgroups(e, aq_cur)
                if e + 1 < E:
                    aq_cur, nd_next = make_nd(e + 1)
                else:
                    nd_next = []
                j = 0
                bias = int(os.environ.get("CODA_BIAS", "2"))
                aqlate = int(os.environ.get("CODA_AQLATE", "1"))
                nlim = len(nd_next) - (1 if (aqlate and nd_next) else 0)
                for i, pg in enumerate(pgs):
                    pg()
                    jt = min(nlim, (i + 1) * bias * len(nd_next) // len(pgs))
                    if aqlate and nd_next and i >= len(pgs) - 2:
                        jt = len(nd_next)
                    while j < jt:
                        nd_next[j]()
                        j += 1

    if not nc.is_finalized():
        nc.finalize()
    return nc


_NC_CACHE = None


def _get_nc():
    global _NC_CACHE
    if _NC_CACHE is None:
        _NC_CACHE = _build_bass()
    return _NC_CACHE


def _prep_inputs(x, Ks, As, Ps):
    x = np.asarray(x, dtype=np.float32)
    Ks = np.asarray(Ks, dtype=np.float32)
    As = np.asarray(As, dtype=np.float32)
    Ps = np.asarray(Ps, dtype=np.float32)

    nrm = np.sqrt(np.sum(Ks * Ks, axis=-1, keepdims=True))
    nK = Ks / np.maximum(nrm, EPS)

    mm_np = mybir.dt.np(MM_DT)
    w2_np = mybir.dt.np(W2_DT)
    ps_np = mybir.dt.np(PS_DT)

    def pack(wT, np_dt, kp=K):
        # [D, E, K] -> SBUF partition-major [128, DC, E, kp] (K zero-padded)
        if kp != K:
            wT = np.concatenate(
                [wT, np.zeros((D, E, kp - K), dtype=wT.dtype)], axis=-1
            )
        return np.ascontiguousarray(
            wT.reshape(DC, 128, E, kp).transpose(1, 0, 2, 3)
        ).astype(np_dt, copy=False)

    w1p = pack((As * nK).transpose(2, 0, 1), mm_np)
    w2p = pack((As * As).transpose(2, 0, 1), w2_np, kp=(128 if DROW else K))

    ps_slices = [
        np.ascontiguousarray(
            Ps[:, :, si * LH : (si + 1) * LH, :].reshape(E, K, NDH)
        ).astype(ps_np, copy=False)
        for si in range(SSPLIT)
    ]
    xT = np.ascontiguousarray(x.T).astype(mm_np, copy=False)  # [D, B]

    in_maps = []
    for c in range(NCORES):
        si, q = divmod(c, QSPLIT)
        in_maps.append(
            {
                "xT": np.ascontiguousarray(xT[:, q * BC : (q + 1) * BC]),
                "w1": w1p,
                "w2": w2p,
                "ps": ps_slices[si],
            }
        )
    return in_maps


def _run(x, Ks, As, Ps, trace=False, **spmd_kwargs):
    nc = _get_nc()
    in_maps = _prep_inputs(x, Ks, As, Ps)
    res = run_bass_kernel_spmd(nc, in_maps, list(range(NCORES)), trace=trace, **spmd_kwargs)
    out = np.empty((2, E, B, L // 2, D), dtype=np.float32)
    for c in range(NCORES):
        si, q = divmod(c, QSPLIT)
        s, lp = divmod(si * LH, L // 2)
        out[s, :, q * BC : (q + 1) * BC, lp : lp + LH] = np.asarray(
            res.results[c]["out"]
        ).astype(np.float32, copy=False)
    return out, res


def kernel(x, Ks, As, Ps):
    out, _ = _run(x, Ks, As, Ps, trace=False)
    return out



# revision 66
# speedup vs baseline: 1.0013x; 1.0013x over previous
"""CodaPrompt kernel for Trainium2 (Bass/Tile) on 8 NeuronCores.

Math (reference):
    a[e,b,k,:] = x[b,:] * As[e,k,:]
    q = a / max(||a||_2, eps)        (normalize over d)
    nK = Ks / max(||Ks||_2, eps)
    aq[e,b,k] = <q[e,b,k,:], nK[e,k,:]>
    P_[e,b,l,:] = sum_k aq[e,b,k] * Ps[e,k,l,:]
    out = stack([P_[:,:, :L/2], P_[:,:, L/2:]])   # [2, E, B, L/2, D]

Sharding: SSPLIT L-slices x (8/SSPLIT) batch-slices (default 2x4). The cost
model serializes ALL DMA on one 360GB/s device, so total bytes/core is the
roofline; the output store dominates (15.7MB in bf16 = 42.7us of the ~58us
total). SSPLIT=2 balances the Ps input duplication (6.14MB/SSPLIT) against
num/den compute duplication (scales with B/QSPLIT): PE work (38us) stays
safely under the DMA floor so the store stream is never PE-paced.

Timeline shape: ~2us DMA pipeline latency, ~14us input loads (during which
PE warms its p-state on dummy matmuls and runs num/den for e=0; x^2 splits
DVE/ACT so neither serial chain gates den), then 60 back-to-back 728ns
output stores (2 psum chunks each — small enough to issue right behind
their own copies, large enough to stay above the ~650ns HWDGE descriptor-
gen floor) with num/den for e+1 interleaved between the P_ store groups of
e. psum->sbuf copies round-robin ACT/DVE (GPSIMD cannot access PSUM).
Measured: 61282ns = 57.6us serial DMA (gapless store stream) + 2.0us DMA
pipeline startup + 1.5us completion-sem tail + ~0.2us slack. Verified HW
rel err: 5.7e-3 (gate 2e-2).

Device-side formulation (per core: batch slice of BC rows, one L-slice):
    num[e,k,b] = sum_d (As*nK)[e,k,d] * x[b,d]        -> matmul, contraction over d
    den2[e,k,b] = sum_d (As*As)[e,k,d] * x2[b,d]      -> matmul (x2 on device)
    aq[e,k,b] = num * rsqrt(den2)                      (ACT sqrt + DVE recip + mul)
    P_half[b, (l d)] = aq[e,:,b].T @ Ps[e, :, half]    -> matmul, contraction over k

Host prep is O(E*K*D) pool preprocessing (normalize Ks, fuse/transpose
weights, slice Ps halves) plus the x transpose; all O(B*...) FLOPs on device.
"""

import os
import sys
from contextlib import ExitStack

import numpy as np

if "/opt/trn_rl_repo" not in sys.path:
    sys.path.insert(0, "/opt/trn_rl_repo")

import concourse.mybir as mybir
from concourse import bacc, tile
from concourse.bass_utils import run_bass_kernel_spmd

B, D, E, K, L = 2048, 768, 5, 100, 8
NCORES = 8
SSPLIT = int(os.environ.get("CODA_SSPLIT", "2"))  # L-axis splits (2 or 4)
QSPLIT = NCORES // SSPLIT # batch splits
BC = B // QSPLIT          # batch rows per core
LH = L // SSPLIT          # l entries per core
DC = D // 128             # 6 contraction chunks of 128
NDH = LH * D              # P_ cols per core
NCHUNK = 512              # psum bank width in f32
NJ = NDH // NCHUNK        # n-chunks per core
MC = BC // 128            # output-partition chunks
MSTORE = int(os.environ.get("CODA_MSTORE", "1"))  # m-chunks per output DMA
JSTORE = int(os.environ.get("CODA_JSTORE", "2"))  # j-chunks per store (MSTORE=1)
NWARM = int(os.environ.get("CODA_NWARM", "40"))   # PE clock-ramp dummy matmuls
AQC = int(os.environ.get("CODA_AQC", "512"))      # aq math column-chunk width
AQC0 = int(os.environ.get("CODA_AQC0", "512"))    # aq chunk width for e=0 (head)
AQORD = int(os.environ.get("CODA_AQORD", "0"))    # 0: sqrt->recip; 1: recip->sqrt
AQPRIO = int(os.environ.get("CODA_AQPRIO", "0"))  # scheduler priority hoist for aq ops
# fp8 DoubleRow den matmuls (0.5 cyc/row): compiles with K padded to 128
# (dual-fp8 ldweights ISA wants full/aligned column blocks) but the pad's
# extra DMA bytes outweigh the PE saving at this sharding — PE is not the
# bottleneck. Kept as an option for PE-bound variants.
DROW = int(os.environ.get("CODA_DROW", "0"))
COPY0 = os.environ.get("CODA_COPY0", "")          # engine override for m=0 copies
NB = max(1, BC // 512)    # moving-operand chunks for num/den (fp32 N<=512)
EPS = 1e-12

F32 = mybir.dt.float32
# The cost model serializes ALL DMA traffic on one exclusive DMA_ENGINES
# device at 360 GB/s (descriptors/16 * elem_bytes/22.5ns), so total bytes
# moved per core IS the roofline. bf16 end-to-end (inputs, matmuls, output
# store, fp32 upcast on host) halves every byte vs fp32/fp32r at identical
# PE cost (1 cycle/row for both bf16 and fp32r at N>=256); measured
# ~2e-3 scale-relative error vs the 2e-2 gate.
MM_DTYPE = os.environ.get("CODA_MM_DTYPE", "bfloat16")
MM_DT = getattr(mybir.dt, MM_DTYPE)
PS_DTYPE = os.environ.get("CODA_PS_DTYPE", MM_DTYPE)
PS_DT = getattr(mybir.dt, PS_DTYPE)
# Output DRAM tensor dtype: bf16 halves the dominant store stream
# (31.5MB -> 15.7MB per core); host upcasts to fp32 after gather.
OUT_DTYPE = os.environ.get("CODA_OUT_DTYPE", "bfloat16")
OUT_DT = getattr(mybir.dt, OUT_DTYPE)
# den = sum_d (As^2)[d] * (x^2)[d] is an all-positive contraction, so fp8
# quantization error averages down (~0.2% on den, ~0.1% on aq after rsqrt):
# carrying W2=As^2 in fp8e4m3 (x^2 is squared on device into fp8) halves
# that weight stream's DMA for negligible error.
W2_DTYPE = os.environ.get("CODA_W2_DTYPE", "float8e4")
W2_DT = getattr(mybir.dt, W2_DTYPE)


def _build_bass(repeat=1):
    # Bacc (not plain Bass): its finalize() runs move_matmul_waits_to_ldweights
    # + generate_event_semaphores, without which multi-dependency matmuls hit
    # walrus "Too many sync wait commands".
    # `repeat` replicates the whole compute body (timing instrumentation:
    # slope over repeat removes per-launch overhead); results are idempotent.
    nc = bacc.Bacc(None)

    # Matmul operands must be produced as MM_DT end-to-end (walrus verifies
    # fp32r consumers see fp32r producers). float32r is bit-identical to
    # float32 in DRAM, so host arrays stay np.float32 either way.
    xT_d = nc.declare_dram_parameter("xT", [D, BC], MM_DT, isOutput=False)
    # w1/w2 pre-packed on host into SBUF partition-major layout so each
    # loads as ONE full-rate DMA (per-partition runs of 6000B / 3000B).
    w1_d = nc.declare_dram_parameter("w1", [128, DC, E, K], MM_DT, isOutput=False)
    # K padded to 128 when DROW: dual-fp8 ldweights ISA restrictions want
    # full/aligned column blocks (zero columns land in den rows 100..127,
    # which nothing reads).
    KP = 128 if DROW else K
    w2_d = nc.declare_dram_parameter("w2", [128, DC, E, KP], W2_DT, isOutput=False)
    ps_d = nc.declare_dram_parameter("ps", [E, K, NDH], PS_DT, isOutput=False)
    out_d = nc.declare_dram_parameter("out", [E, BC, LH, D], OUT_DT, isOutput=True)

    with ExitStack() as ctx:
        tc = ctx.enter_context(tile.TileContext(nc))
        const = ctx.enter_context(tc.tile_pool(name="const", bufs=1))
        psp = ctx.enter_context(tc.tile_pool(name="psp", bufs=E))
        smallp = ctx.enter_context(tc.tile_pool(name="smallp", bufs=2))
        resp = ctx.enter_context(
            tc.tile_pool(name="resp", bufs=int(os.environ.get("CODA_RESP", "8")))
        )
        # num/den psum tiles span ceil(BC*4B/2KB) banks; keep total <= 8.
        pndp = ctx.enter_context(
            tc.tile_pool(name="pndp", bufs=(2 if BC <= 512 else 1), space="PSUM")
        )
        ppp = ctx.enter_context(tc.tile_pool(name="ppp", bufs=4, space="PSUM"))

        # Resident operands: x slice (transposed) and the fused W1=As*nK /
        # W2=As^2 weight block, chunked to 128 partitions. Per-chunk loads so
        # the first num/den matmuls start as soon as their own d-chunk lands.
        # x^2 is computed on-device (saves its DMA).
        xT_r = xT_d[:].rearrange("(c p) b -> p c b", p=128)
        xs = const.tile([128, DC, BC], MM_DT, name="xs", tag="xs")
        x2s = const.tile([128, DC, BC], W2_DT, name="x2s", tag="x2s")
        ws1 = const.tile([128, DC, E, K], MM_DT, name="ws1", tag="ws1")
        ws2 = const.tile([128, DC, E, KP], W2_DT, name="ws2", tag="ws2")
        # PE p-state warmup: the cost model charges matmuls ~2x cycles until
        # the PE has been continuously busy ~3us. Dummy matmuls on a zeroed
        # scratch tile ramp the clock while the first loads are in flight,
        # so the real num/den matmuls all run at full rate. Results land in
        # rotating psum scratch that is never read.
        if NWARM:
            warm = const.tile([128, 128], MM_DT, name="warm", tag="warm")
            nc.gpsimd.memset(warm[:], 0)
            for _ in range(NWARM):
                wp = ppp.tile([128, NCHUNK], F32, name="pp", tag="pp")
                nc.tensor.matmul(wp[:, :128], warm[:], warm[:], start=True, stop=True)

        # Few, large loads: every transfer stays above the ~650ns HWDGE
        # descriptor-gen serialization (per-chunk ws/xs loads leak ~225ns of
        # dead DMA time each). ws1 in two halves so the first num matmuls
        # start while the second half is in flight.
        nc.sync.dma_start(xs[:], xT_r[:])
        for c0 in range(0, DC, 2):
            nc.sync.dma_start(ws1[:, c0 : c0 + 2], w1_d[:, c0 : c0 + 2])
        nc.sync.dma_start(ws2[:], w2_d[:])
        # x^2 split DVE/ACT: the fp8 output disables DVE's 2x mode (594ns
        # per chunk), and a serial 6-chunk chain on one engine gates den(e0).
        # Both engines are idle during the load phase.
        for c in range(DC):
            if c % 2 == 0:
                nc.vector.tensor_mul(x2s[:, c], xs[:, c], xs[:, c])
            else:
                nc.scalar.square(x2s[:, c], xs[:, c])

        # psum->sbuf copy engines round-robin DVE / ACT. (GPSIMD/Pool cannot
        # access PSUM on TRN2 — the walrus verifier rejects it.) The copies
        # are one full pass over the output; the 2-way split keeps both
        # engines under the serial-DMA floor.
        copy_engines = os.environ.get("CODA_COPY_ENGINES", "sv")
        copy_ctr = [0]

        def emit_copy(dst, src, eng=None):
            if eng is None:
                eng = copy_engines[copy_ctr[0] % len(copy_engines)]
                copy_ctr[0] += 1
            if eng == "v":
                nc.vector.tensor_copy(dst, src)
            else:
                nc.scalar.copy(dst, src)

        for _ in range(repeat):
            # All pool loads issue upfront (own slots, bufs=E) so no load
            # ever queues behind output stores in a DMA FIFO. (Staggering the
            # last loads to fill the store-rampup bubble does not work: the
            # scheduler hoists dependency-free DMAs.)
            psts = []
            for e in range(E):
                pst = psp.tile([K, NDH], PS_DT, name="pst", tag="ps")
                nc.sync.dma_start(pst[:], ps_d[e])
                psts.append(pst)

            def make_nd(e, aqc=AQC):
                """aq tile + list of emit-steps (num+den matmuls per d-chunk;
                aq math attached after the last chunk). `aqc` sets the aq
                column-chunk width: fine chunks for e=0 cut the first-store
                latency (P_ m0 only needs aq[:, :128]); coarse elsewhere
                avoids per-op overhead."""
                num = pndp.tile([K, BC], F32, name="num", tag="num")
                den = pndp.tile([KP, BC], F32, name="den", tag="den")
                sden = smallp.tile([K, BC], F32, name="sden", tag="sden")
                rden = smallp.tile([K, BC], F32, name="rden", tag="rden")
                aq = smallp.tile([K, BC], PS_DT, name="aq", tag="aq", bufs=2)
                steps = []
                for nb in range(NB):
                    bsl = slice(nb * 512, min((nb + 1) * 512, BC))

                    def mk(c, bsl=bsl, last=False):
                        # num+den for one d-chunk per step: den finishes right
                        # behind the last weight-chunk load instead of a full
                        # num pass later (shorter first-aq critical path).
                        def emit():
                            nc.tensor.matmul(
                                num[:, bsl],
                                ws1[:, c, e, :],
                                xs[:, c, bsl],
                                start=(c == 0),
                                stop=(c == DC - 1),
                            )
                            if DROW:
                                # fp8 DoubleRow: 2 d-chunks per pass at 0.5
                                # cycles/row — the [128, DC, ...] tiles put
                                # the k-tile pair exactly at AP dim 1.
                                if c % 2 == 1:
                                    nc.tensor.matmul(
                                        den[:, bsl],
                                        ws2[:, c - 1 : c + 1, e, :],
                                        x2s[:, c - 1 : c + 1, bsl],
                                        start=(c == 1),
                                        stop=(c == DC - 1),
                                        perf_mode=mybir.MatmulPerfMode.DoubleRow,
                                    )
                            else:
                                nc.tensor.matmul(
                                    den[:, bsl],
                                    ws2[:, c, e, :],
                                    x2s[:, c, bsl],
                                    start=(c == 0),
                                    stop=(c == DC - 1),
                                )
                            if last:
                                # aq = num / sqrt(den2) (den2 >> eps^2 here),
                                # in column chunks: P_ for m-chunk q only
                                # needs aq[:, q*128:...], so finer chunks cut
                                # the first-store latency and the e-boundary
                                # stall where P_(e+1) waits on the aq chain.
                                b0, b1 = bsl.start, bsl.stop
                                ctx2 = (
                                    tc.high_priority(offset=AQPRIO)
                                    if AQPRIO
                                    else None
                                )
                                if ctx2:
                                    ctx2.__enter__()
                                for q0 in range(b0, b1, aqc):
                                    qsl = slice(q0, min(q0 + aqc, b1))
                                    if AQORD == 0:
                                        nc.scalar.sqrt(sden[:, qsl], den[:K, qsl])
                                        nc.vector.reciprocal(rden[:, qsl], sden[:, qsl])
                                    else:
                                        # same math, DVE->ACT->DVE shape:
                                        # rsqrt(den) = sqrt(1/den)
                                        nc.vector.reciprocal(sden[:, qsl], den[:K, qsl])
                                        nc.scalar.sqrt(rden[:, qsl], sden[:, qsl])
                                    nc.vector.tensor_mul(
                                        aq[:, qsl], num[:, qsl], rden[:, qsl]
                                    )
                                if ctx2:
                                    ctx2.__exit__(None, None, None)

                        return emit

                    for c in range(DC):
                        steps.append(mk(c, last=(c == DC - 1)))
                return aq, steps

            def make_pgroups(e, aq):
                """MC emit-steps: NJ P_ matmuls + copies per m-chunk; one DMA
                per MSTORE m-chunks (fewer, larger stores -> fewer HWDGE gens
                and completion-sem boundaries on the store stream)."""
                pst = psts[e]
                out_r = out_d[e].rearrange("b l d -> b (l d)")
                pgs = []
                res_holder = [None]
                for m in range(MC):
                    def emit(m=m):
                        g = m % MSTORE
                        if g == 0:
                            res_holder[0] = resp.tile(
                                [128, MSTORE * NDH], OUT_DT, name="res", tag="res"
                            )
                        res = res_holder[0]
                        # JSTORE j-chunks per store DMA: finer stores issue
                        # as soon as their own copies land (shorter first-
                        # store latency, smoother stream); transfers must
                        # stay >= the ~650ns HWDGE gen time.
                        for j in range(NJ):
                            pp = ppp.tile([128, NCHUNK], F32, name="pp", tag="pp")
                            nc.tensor.matmul(
                                pp[:],
                                aq[:, m * 128 : (m + 1) * 128],
                                pst[:, j * NCHUNK : (j + 1) * NCHUNK],
                                start=True,
                                stop=True,
                            )
                            # First m-chunk after an e-boundary: copies go to
                            # ACT only, freeing DVE for the aq chain burst
                            # (recip+mul) that lands there.
                            emit_copy(
                                res[:, g * NDH + j * NCHUNK : g * NDH + (j + 1) * NCHUNK],
                                pp[:],
                                eng=(COPY0 if (m == 0 and COPY0) else None),
                            )
                            if MSTORE == 1 and (j + 1) % JSTORE == 0:
                                j0 = j + 1 - JSTORE
                                nc.sync.dma_start(
                                    out_r[m * 128 : (m + 1) * 128][
                                        :, j0 * NCHUNK : (j + 1) * NCHUNK
                                    ],
                                    res[:, j0 * NCHUNK : (j + 1) * NCHUNK],
                                )
                        if MSTORE > 1 and g == MSTORE - 1:
                            m0 = m - g
                            dst = out_r[m0 * 128 : (m0 + MSTORE) * 128].rearrange(
                                "(g p) n -> p g n", p=128
                            )
                            nc.sync.dma_start(dst, res[:].rearrange(
                                "p (g n) -> p g n", g=MSTORE
                            ))

                    pgs.append(emit)
                return pgs

            # Software pipeline: num/den for e+1 interleaves with the P_
            # store-groups of e, so PE keeps feeding the store stream instead
            # of stalling DMA for ~5us per layer during the nd phase. The
            # interleave is front-biased (factor 2): nd work lands while the
            # store buffer is still full, leaving a pure-P_ tail that streams
            # stores at full DMA rate.
            aq_cur, nd_steps = make_nd(0, aqc=AQC0)
            for s in nd_steps:
                s()
            for e in range(E):
                pgs = make_pgroups(e, aq_cur)
                if e + 1 < E:
                    aq_cur, nd_next = make_nd(e + 1)
                else:
                    nd_next = []
                j = 0
                bias = int(os.environ.get("CODA_BIAS", "2"))
                aqlate = int(os.environ.get("CODA_AQLATE", "1"))
                nlim = len(nd_next) - (1 if (aqlate and nd_next) else 0)
                for i, pg in enumerate(pgs):
                    pg()
                    jt = min(nlim, (i + 1) * bias * len(nd_next) // len(pgs))
                    if aqlate and nd_next and i >= len(pgs) - 2:
                        jt = len(nd_next)
                    while j < jt:
                        nd_next[j]()
                        j += 1

    if not nc.is_finalized():
        nc.finalize()
    return nc


_NC_CACHE = None


def _get_nc():
    global _NC_CACHE
    if _NC_CACHE is None:
        _NC_CACHE = _build_bass()
    return _NC_CACHE


def _prep_inputs(x, Ks, As, Ps):
    x = np.asarray(x, dtype=np.float32)
    Ks = np.asarray(Ks, dtype=np.float32)
    As = np.asarray(As, dtype=np.float32)
    Ps = np.asarray(Ps, dtype=np.float32)

    nrm = np.sqrt(np.sum(Ks * Ks, axis=-1, keepdims=True))
    nK = Ks / np.maximum(nrm, EPS)

    mm_np = mybir.dt.np(MM_DT)
    w2_np = mybir.dt.np(W2_DT)
    ps_np = mybir.dt.np(PS_DT)

    def pack(wT, np_dt, kp=K):
        # [D, E, K] -> SBUF partition-major [128, DC, E, kp] (K zero-padded)
        if kp != K:
            wT = np.concatenate(
                [wT, np.zeros((D, E, kp - K), dtype=wT.dtype)], axis=-1
            )
        return np.ascontiguousarray(
            wT.reshape(DC, 128, E, kp).transpose(1, 0, 2, 3)
        ).astype(np_dt, copy=False)

    w1p = pack((As * nK).transpose(2, 0, 1), mm_np)
    w2p = pack((As * As).transpose(2, 0, 1), w2_np, kp=(128 if DROW else K))

    ps_slices = [
        np.ascontiguousarray(
            Ps[:, :, si * LH : (si + 1) * LH, :].reshape(E, K, NDH)
        ).astype(ps_np, copy=False)
        for si in range(SSPLIT)
    ]
    xT = np.ascontiguousarray(x.T).astype(mm_np, copy=False)  # [D, B]

    in_maps = []
    for c in range(NCORES):
        si, q = divmod(c, QSPLIT)
        in_maps.append(
            {
                "xT": np.ascontiguousarray(xT[:, q * BC : (q + 1) * BC]),
                "w1": w1p,
                "w2": w2p,
                "ps": ps_slices[si],
            }
        )
    return in_maps


def _run(x, Ks, As, Ps, trace=False, **spmd_kwargs):
    nc = _get_nc()
    in_maps = _prep_inputs(x, Ks, As, Ps)
    res = run_bass_kernel_spmd(nc, in_maps, list(range(NCORES)), trace=trace, **spmd_kwargs)
    out = np.empty((2, E, B, L // 2, D), dtype=np.float32)
    for c in range(NCORES):
        si, q = divmod(c, QSPLIT)
        s, lp = divmod(si * LH, L // 2)
        out[s, :, q * BC : (q + 1) * BC, lp : lp + LH] = np.asarray(
            res.results[c]["out"]
        ).astype(np.float32, copy=False)
    return out, res


def kernel(x, Ks, As, Ps):
    out, _ = _run(x, Ks, As, Ps, trace=False)
    return out

